# revision 4
# baseline (speedup 1.0000x reference)
"""GQA attention (B=2,S=1024,HID=2048,NH=32,NKV=8,HD=64) on 8 TRN2 cores — v4.

Sharding: core c -> batch b=c//4, head-group g=c%4 (8 q heads / 2 kv heads).
Per core: partial out[b] = attn(heads of g) @ Wo[rows of g]; host sums the 4
row-parallel bf16 partials per batch.

The exp stream on ACT (64 x [128,1024] ~= 66us) and total PE work (~125us)
are the two walls; the schedule keeps both streams dense:

  phase A: ONE k-chunk-outer chase loop computes kv-proj, natural-layout V,
           AND q-chunk mc0 per hsT chunk (12 matmuls ~= 1.3us PE per 1.06us
           of DMA), fed by a host-packed per-chunk weight block
           [wk_k|wv_k|wq0_k] interleaved with the hsT chunks in DMA order.
           Then mc1 + the kv/mc0 rope rotations interleaved.
  phase B: attention slot stream (h,kc): sc(i) -> exp(i) -> fill -> pv(i-1).
           scores psum bufs=2 keeps the exp cadence gapless; PV lags one
           slot so it never head-of-line blocks the PE behind an exp wait;
           rope(mc1) + mc2/mc3 projections + their ropes are drip-fed as
           filler (~0.9 steps/slot). ps_at (PV accum, bufs=1) is drained to
           SBUF by a Pool-engine copy (DVE is busy; Pool is the fast lane)
           right after pv(h,7); softmax normalize (recip -> row-0 hop DMA ->
           gpsimd broadcast -> mult) runs off-psum. All heads use augmented
           [V|1] PV weights: denominators land on psum row 64 (no
           ones-matmuls). Odd heads reach attnT rows 64..127 via SBUF DMA.
  phase C: Wo with kc2=3 (head 6/7 rows) emitted two mc2-passes late so the
           head-7 normalize hides under the first Wo passes; psum->SBUF
           output copies alternate ACT/DVE; bf16 output halves the out-DMA.

PSUM banks: A: pA 3x[128,1024] + rot 1x = 8; B: stp 2x + avp 1x + pB 1x = 8;
C: wop 3x = 6.
"""

import numpy as np
import ml_dtypes

import concourse.bass as bass
import concourse.bacc as bacc
import concourse.mybir as mybir
from concourse.tile import TileContext
from concourse.bass_utils import run_bass_kernel_spmd
from concourse.masks import make_identity

B, S, HID = 2, 1024, 2048
NH, NKV, HD = 32, 8, 64
G = 4                      # head groups (tensor-parallel degree per batch)
QH = NH // G               # 8 q heads per core
KVH = NKV // G             # 2 kv heads per core
QD = QH * HD               # 512
HC = HID // 128            # 16 hidden chunks
TC = S // 128              # 8 token chunks
KC = S // 128              # 8 k chunks
ROPE_BASE = 10000.0
BF16 = mybir.dt.bfloat16
F32 = mybir.dt.float32
NEG_BIG = float(np.finfo(np.float32).min)

LAST_RESULT = None
_CACHE = {}


def _build(use_mask: bool) -> bass.Bass:
    nc = bacc.Bacc(None, target_bir_lowering=False)
    hsT_d = nc.dram_tensor("hsT", [HID, S], BF16, kind="ExternalInput")
    # per-chunk weight block: [wk_k (128) | wv_k (128) | wq_mc0_k (128)]
    wch_d = nc.dram_tensor("wch", [128, HC * 384], BF16, kind="ExternalInput")
    wq_d = nc.dram_tensor("wq2", [128, 3 * HC * 128], BF16, kind="ExternalInput")
    wo_d = nc.dram_tensor("wo2", [128, 4 * HC * 128], BF16, kind="ExternalInput")
    cos_d = nc.dram_tensor("cos2", [128, S], BF16, kind="ExternalInput")
    sin_d = nc.dram_tensor("sin2", [128, S], BF16, kind="ExternalInput")
    perm_d = nc.dram_tensor("permT", [128, 128], BF16, kind="ExternalInput")
    if use_mask:
        mask_d = nc.dram_tensor("maskT", [S, S], BF16, kind="ExternalInput")
    out_d = nc.dram_tensor("out", [HID, S], BF16, kind="ExternalOutput")

    with TileContext(nc) as tc:
        with (
            tc.tile_pool(name="resid", bufs=1) as rp,
            tc.tile_pool(name="ex", bufs=3) as ep,
            tc.tile_pool(name="wk_", bufs=2) as wp,
            tc.tile_pool(name="outs", bufs=4) as op_,
        ):
            # ---- resident SBUF tiles + input DMA (SP order == emission) ----
            wch = rp.tile([128, HC * 384], BF16, tag="wch")
            hsT = []
            for k in range(HC):
                nc.sync.dma_start(out=wch[:, k * 384:(k + 1) * 384],
                                  in_=wch_d[:, k * 384:(k + 1) * 384])
                t = rp.tile([128, S], BF16, tag=f"hsT{k}")
                nc.sync.dma_start(out=t[:], in_=hsT_d[k * 128:(k + 1) * 128, :])
                hsT.append(t)
            wq2 = rp.tile([128, 3 * HC * 128], BF16, tag="wq2")
            nc.sync.dma_start(out=wq2[:, 0:2048], in_=wq_d[:, 0:2048])
            permT = rp.tile([128, 128], BF16, tag="permT")
            nc.sync.dma_start(out=permT[:], in_=perm_d[:, :])
            cos2 = rp.tile([128, S], BF16, tag="cos2")
            nc.sync.dma_start(out=cos2[:], in_=cos_d[:, :])
            sin2 = rp.tile([128, S], BF16, tag="sin2")
            nc.sync.dma_start(out=sin2[:], in_=sin_d[:, :])
            nc.sync.dma_start(out=wq2[:, 2048:4096], in_=wq_d[:, 2048:4096])
            nc.sync.dma_start(out=wq2[:, 4096:6144], in_=wq_d[:, 4096:6144])
            if use_mask:
                maskT = rp.tile([128, KC * S], BF16, tag="maskT")
                nc.sync.dma_start(
                    out=maskT[:].rearrange("p (k q) -> p k q", k=KC),
                    in_=mask_d[:, :].rearrange("(k p) q -> p k q", p=128),
                )
            wo2 = rp.tile([128, 4 * HC * 128], BF16, tag="wo2")
            for i in range(4):
                nc.sync.dma_start(out=wo2[:, i * 2048:(i + 1) * 2048],
                                  in_=wo_d[:, i * 2048:(i + 1) * 2048])

            # ---- persistent intermediates ----
            qrot = rp.tile([128, 4 * S], BF16, tag="qrot")
            krot = rp.tile([128, S], BF16, tag="krot")
            krep = rp.tile([128, KVH * S], BF16, tag="krep")
            # vaug per (kc, kv): 128 cols [1 | 0*63 | V(64)] — PV lhsT puts
            # the softmax denominator on psum row 0 (so partition_broadcast
            # reads partition 0, the HW-supported source) and attn on rows
            # 64..127 (a DVE-legal 64-partition window).
            vaug = rp.tile([128, KC * KVH * 128], BF16, tag="vaug")
            nc.any.memset(vaug[:], 0.0)
            ones_cols = vaug[:].rearrange("p (b c) -> p b c", c=128)[:, :, 0:1]
            nc.vector.memset(ones_cols, 1.0)
            attnT = [rp.tile([128, S], BF16, tag=f"attnT{i}", name=f"attnT{i}")
                     for i in range(4)]

            # preload the exp activation table while ACT is idle
            junk = rp.tile([1, 4], F32, tag="junk")
            nc.vector.memset(junk[:], 0.0)
            nc.scalar.activation(
                junk[0:1, 2:4], junk[0:1, 0:2],
                mybir.ActivationFunctionType.Exp
            )

            t1 = rp.tile([128, S], BF16, tag="t1")
            t2 = rp.tile([128, S], BF16, tag="t2")
            ident = rp.tile([128, 128], BF16, tag="ident")
            make_identity(nc, ident[:])

            def rope_combine(raw, ps_rot, dst):
                """dst = raw*cos + rot(raw)*sin, all [128, S] on DVE."""
                nc.vector.tensor_tensor(t1[:], raw[:], cos2[:],
                                        mybir.AluOpType.mult)
                nc.vector.tensor_tensor(t2[:], ps_rot[:], sin2[:],
                                        mybir.AluOpType.mult)
                nc.vector.tensor_tensor(dst, t1[:], t2[:], mybir.AluOpType.add)

            # ===== phase A: kv+v+mc0 in one DMA-chased loop, then mc1 =====
            with (
                tc.tile_pool(name="pA", bufs=3, space="PSUM") as pA,
                tc.tile_pool(name="rot", bufs=1, space="PSUM") as rot_p,
            ):
                ps_kv = pA.tile([128, S], F32, tag="pA")
                ps_vt = pA.tile([128, S], F32, tag="pA")
                ps_q0 = pA.tile([128, S], F32, tag="pA")
                for k in range(HC):
                    wb = k * 384
                    for ns in range(2):
                        nc.tensor.matmul(
                            ps_kv[:, ns * 512:(ns + 1) * 512],
                            wch[:, wb:wb + 128],
                            hsT[k][:, ns * 512:(ns + 1) * 512],
                            start=(k == 0), stop=(k == HC - 1),
                        )
                    for ns in range(2):
                        nc.tensor.matmul(
                            ps_vt[:, ns * 512:(ns + 1) * 512],
                            wch[:, wb + 128:wb + 256],
                            hsT[k][:, ns * 512:(ns + 1) * 512],
                            start=(k == 0), stop=(k == HC - 1),
                        )
                    for ns in range(2):
                        nc.tensor.matmul(
                            ps_q0[:, ns * 512:(ns + 1) * 512],
                            wch[:, wb + 256:wb + 384],
                            hsT[k][:, ns * 512:(ns + 1) * 512],
                            start=(k == 0), stop=(k == HC - 1),
                        )

                # psum -> sbuf raws on ACT (run during mc1 below)
                kraw = rp.tile([128, S], BF16, tag="kraw")
                nc.scalar.activation(
                    kraw[:], ps_kv[:], mybir.ActivationFunctionType.Copy
                )
                q0raw = rp.tile([128, S], BF16, tag="q0raw")
                nc.scalar.activation(
                    q0raw[:], ps_q0[:], mybir.ActivationFunctionType.Copy
                )
                vt_sb = rp.tile([128, S], BF16, tag="vt_sb")
                nc.scalar.activation(
                    vt_sb[:], ps_vt[:], mybir.ActivationFunctionType.Copy
                )

                # mc1 projection, with the kv/mc0 rope rotations interleaved
                ps_q1 = pA.tile([128, S], F32, tag="pA")

                def mc1_half(lo, hi):
                    for k in range(lo, hi):
                        for ns in range(2):
                            nc.tensor.matmul(
                                ps_q1[:, ns * 512:(ns + 1) * 512],
                                wq2[:, k * 128:(k + 1) * 128],
                                hsT[k][:, ns * 512:(ns + 1) * 512],
                                start=(k == 0), stop=(k == HC - 1),
                            )

                mc1_half(0, 4)
                ps_r0 = rot_p.tile([128, S], F32, tag="rot")
                for ns in range(2):
                    nc.tensor.matmul(
                        ps_r0[:, ns * 512:(ns + 1) * 512],
                        permT[:], q0raw[:, ns * 512:(ns + 1) * 512],
                        start=True, stop=True,
                    )
                rope_combine(q0raw, ps_r0, qrot[:, 0:S])
                mc1_half(4, 6)
                ps_rk = rot_p.tile([128, S], F32, tag="rot")
                for ns in range(2):
                    nc.tensor.matmul(
                        ps_rk[:, ns * 512:(ns + 1) * 512],
                        permT[:], kraw[:, ns * 512:(ns + 1) * 512],
                        start=True, stop=True,
                    )
                rope_combine(kraw, ps_rk, krot[:])
                # krep: kv head i duplicated into both 64-row halves
                nc.vector.tensor_copy(krep[0:64, 0:S], krot[0:64, :])
                nc.vector.tensor_copy(krep[64:128, S:2 * S], krot[64:128, :])
                nc.sync.dma_start(out=krep[64:128, 0:S], in_=krot[0:64, :])
                nc.sync.dma_start(out=krep[0:64, S:2 * S], in_=krot[64:128, :])
                mc1_half(6, 11)
                # PE-transpose V^T chunks into natural [token, dim] layout;
                # each transpose is its own start+stop group (bank-legal)
                trt = pA.tile([128, 2 * S], BF16, tag="pA", name="trt")
                for t in range(TC):
                    nc.tensor.transpose(
                        trt[:, t * 128:(t + 1) * 128],
                        vt_sb[:, t * 128:(t + 1) * 128], ident[:]
                    )
                mc1_half(11, HC)

                # vaug: V cols at base+64..base+127 per (t, kv)
                for t in range(TC):
                    for kv in range(KVH):
                        base = (t * KVH + kv) * 128
                        nc.vector.tensor_copy(
                            vaug[:, base + 64:base + 128],
                            trt[:, t * 128 + kv * 64:t * 128 + kv * 64 + 64],
                        )

                q1raw = rp.tile([128, S], BF16, tag="q1raw")
                nc.scalar.activation(
                    q1raw[:], ps_q1[:], mybir.ActivationFunctionType.Copy
                )

            # ===== phase B: attention slot stream with rope(mc1)+mc2/mc3 fill
            with (
                tc.tile_pool(name="st", bufs=2, space="PSUM") as stp,
                tc.tile_pool(name="av", bufs=1, space="PSUM") as avp,
                tc.tile_pool(name="pB", bufs=1, space="PSUM") as pBp,
            ):
                def fill_steps():
                    # rope(mc1) first: needed by h2 (slot 16)
                    ps_r1 = pBp.tile([128, S], F32, tag="pB")
                    for ns in range(2):
                        nc.tensor.matmul(
                            ps_r1[:, ns * 512:(ns + 1) * 512],
                            permT[:], q1raw[:, ns * 512:(ns + 1) * 512],
                            start=True, stop=True,
                        )
                    yield
                    rope_combine(q1raw, ps_r1, qrot[:, S:2 * S])
                    yield
                    # deferred mc2/mc3 projection + rope, ~2 matmuls/step
                    for mc in (2, 3):
                        ps_q = pBp.tile([128, S], F32, tag="pB")
                        for k in range(HC):
                            for ns in range(2):
                                nc.tensor.matmul(
                                    ps_q[:, ns * 512:(ns + 1) * 512],
                                    wq2[:, ((mc - 1) * HC + k) * 128:
                                        ((mc - 1) * HC + k + 1) * 128],
                                    hsT[k][:, ns * 512:(ns + 1) * 512],
                                    start=(k == 0), stop=(k == HC - 1),
                                )
                            yield
                        qraw = rp.tile([128, S], BF16, tag=f"qraw{mc}")
                        nc.vector.tensor_copy(qraw[:], ps_q[:])  # ACT is busy
                        yield
                        ps_rq = pBp.tile([128, S], F32, tag="pB")
                        for ns in range(2):
                            nc.tensor.matmul(
                                ps_rq[:, ns * 512:(ns + 1) * 512],
                                permT[:], qraw[:, ns * 512:(ns + 1) * 512],
                                start=True, stop=True,
                            )
                        yield
                        rope_combine(qraw, ps_rq, qrot[:, mc * S:(mc + 1) * S])
                        yield

                fill_gen = fill_steps()

                def fill(n=1):
                    nonlocal fill_gen
                    for _ in range(n):
                        if fill_gen is None:
                            return
                        try:
                            next(fill_gen)
                        except StopIteration:
                            fill_gen = None
                            return

                def emit_sc(h, kc):
                    kv = h // 4
                    mc = h // 2
                    r = (h % 2) * 64
                    ps_st = stp.tile([128, S], F32, tag="st")
                    for ns in range(2):
                        nc.tensor.matmul(
                            ps_st[:, ns * 512:(ns + 1) * 512],
                            krep[r:r + 64,
                                 kv * S + kc * 128:kv * S + (kc + 1) * 128],
                            qrot[r:r + 64,
                                 mc * S + ns * 512:mc * S + ns * 512 + 512],
                            start=True, stop=True,
                        )
                    if use_mask:
                        nc.vector.tensor_tensor(
                            ps_st[:], ps_st[:], maskT[:, kc * S:(kc + 1) * S],
                            mybir.AluOpType.add,
                        )
                    ex = ep.tile([128, S], BF16, tag="ex")
                    nc.scalar.activation(
                        ex[:], ps_st[:], mybir.ActivationFunctionType.Exp
                    )
                    return ex

                at_tiles = {}

                def emit_pv(h, kc, ex):
                    kv = h // 4
                    if kc == 0:
                        at_tiles[h] = avp.tile([128, S], F32, tag="av",
                                               name=f"av{h}")
                    ps_at = at_tiles[h]
                    base = (kc * KVH + kv) * 65
                    for ns in range(2):
                        nc.tensor.matmul(
                            ps_at[0:65, ns * 512:(ns + 1) * 512],
                            vaug[:, base:base + 65],
                            ex[:, ns * 512:(ns + 1) * 512],
                            start=(kc == 0), stop=(kc == KC - 1),
                        )

                def emit_norm(h):
                    """Drain ps_at to SBUF on Pool (fast lane), normalize
                    off-psum: recip -> row-0 hop -> broadcast -> mult."""
                    mc = h // 2
                    ps_at = at_tiles.pop(h)
                    atc = wp.tile([128, S], F32, tag="atc")
                    nc.gpsimd.tensor_copy(atc[0:65, :], ps_at[0:65, :])
                    nc.vector.reciprocal(atc[64:65, :], atc[64:65, :])
                    bcast = wp.tile([128, S], F32, tag="bcast")
                    nc.gpsimd.partition_broadcast(bcast[0:64, :], atc[64:65, :])
                    if h % 2 == 0:
                        nc.vector.tensor_tensor(
                            attnT[mc][0:64, :],
                            atc[0:64, :], bcast[0:64, :],
                            mybir.AluOpType.mult,
                        )
                    else:
                        tmp = wp.tile([128, S], BF16, tag="otmp")
                        nc.vector.tensor_tensor(
                            tmp[0:64, :], atc[0:64, :], bcast[0:64, :],
                            mybir.AluOpType.mult,
                        )
                        nc.sync.dma_start(
                            out=attnT[mc][64:128, :],
                            in_=tmp[0:64, :],
                        )

                slots = [(h, kc) for h in range(QH) for kc in range(KC)]
                prev = None
                for i, (h, kc) in enumerate(slots):
                    ex = emit_sc(h, kc)
                    if i % 8 != 7:
                        fill(1)
                    if prev is not None:
                        emit_pv(*prev)
                        if prev[1] == KC - 1:
                            emit_norm(prev[0])
                    prev = (h, kc, ex)
                emit_pv(*prev)
                emit_norm(prev[0])
                fill(100)

            # ===== phase C: output projection (transposed out, bf16) =====
            # kc2=3 (heads 6/7) is emitted two mc2-passes late so the first Wo
            # passes overlap the head-7 normalize chain.
            with tc.tile_pool(name="wop", bufs=3, space="PSUM") as wop:
                pso_t = {}

                def wo_part(mc2, kc2s, start, stop):
                    if mc2 not in pso_t:
                        pso_t[mc2] = wop.tile([128, S], F32, tag="wop",
                                              name=f"wop{mc2}")
                    pso = pso_t[mc2]
                    for ns in range(2):
                        for kc2 in kc2s:
                            nc.tensor.matmul(
                                pso[:, ns * 512:(ns + 1) * 512],
                                wo2[:, (kc2 * HC + mc2) * 128:
                                    (kc2 * HC + mc2 + 1) * 128],
                                attnT[kc2][:, ns * 512:ns * 512 + 512],
                                start=(start and kc2 == kc2s[0]),
                                stop=(stop and kc2 == kc2s[-1]),
                            )

                def wo_finish(mc2):
                    wo_part(mc2, [3], start=False, stop=True)
                    pso = pso_t.pop(mc2)
                    osb = op_.tile([128, S], BF16, tag="osb")
                    nc.scalar.activation(
                        osb[:], pso[:], mybir.ActivationFunctionType.Copy
                    )
                    nc.sync.dma_start(
                        out=out_d[mc2 * 128:(mc2 + 1) * 128, :], in_=osb[:]
                    )

                NW = HID // 128
                for mc2 in range(NW):
                    if mc2 != 0:   # mc2=0's kc2 0..2 was parked in phase B
                        wo_part(mc2, [0, 1, 2], start=True, stop=False)
                    if mc2 >= 2:
                        wo_finish(mc2 - 2)
                wo_finish(NW - 2)
                wo_finish(NW - 1)
    nc.finalize()
    return nc


def _rope_tables():
    inv = 1.0 / (ROPE_BASE ** (np.arange(0, HD, 2, dtype=np.float32) / HD))
    t = np.arange(S, dtype=np.float32)
    freqs = np.outer(t, inv)
    emb = np.concatenate([freqs, freqs], axis=-1)  # [S, HD]
    return np.cos(emb).astype(np.float32), np.sin(emb).astype(np.float32)


def _perm_T():
    P = np.zeros((128, 128), dtype=np.float32)
    for blk in range(2):
        o = blk * 64
        for i in range(32):
            P[o + i, o + i + 32] = -1.0
            P[o + i + 32, o + i] = 1.0
    return P.T.astype(ml_dtypes.bfloat16)


def kernel(hidden_states, position_ids, attention_mask, Wq, Wk, Wv, Wo,
           _trace=False):
    global LAST_RESULT
    bf = ml_dtypes.bfloat16
    hidden_states = np.asarray(hidden_states, dtype=np.float32)
    Wq = np.asarray(Wq, dtype=np.float32)
    Wk = np.asarray(Wk, dtype=np.float32)
    Wv = np.asarray(Wv, dtype=np.float32)
    Wo = np.asarray(Wo, dtype=np.float32)
    mask = np.asarray(attention_mask, dtype=np.float32)
    pos = np.asarray(position_ids).astype(np.int64)

    use_mask = bool(np.any(mask))
    if use_mask not in _CACHE:
        _CACHE[use_mask] = _build(use_mask)
    nc = _CACHE[use_mask]

    cos_t, sin_t = _rope_tables()
    permT = _perm_T()
    scale = 1.0 / np.sqrt(HD)

    in_maps = []
    for c in range(8):
        b, g = c // G, c % G
        wq_g = (Wq[:, g * QD:(g + 1) * QD] * scale).astype(np.float32)
        # [k, p, mc, c] -> per-chunk [p, k, (wk|wv|wq0)] and mc-major rest
        wq4 = wq_g.reshape(HC, 128, 4, 128)
        wk3 = Wk[:, g * 128:(g + 1) * 128].reshape(HC, 128, 128)
        wv3 = Wv[:, g * 128:(g + 1) * 128].reshape(HC, 128, 128)
        wch = np.concatenate([wk3, wv3, wq4[:, :, 0]], axis=2)  # [k, p, 384]
        wch = wch.transpose(1, 0, 2).reshape(128, HC * 384)
        wq2 = wq4[:, :, 1:].transpose(1, 2, 0, 3).reshape(128, 3 * HC * 128)
        wo2 = Wo[g * QD:(g + 1) * QD, :].reshape(4, 128, HC, 128).transpose(1, 0, 2, 3)
        m = {
            "hsT": np.ascontiguousarray(hidden_states[b].T).astype(bf),
            "wch": np.ascontiguousarray(wch).astype(bf),
            "wq2": np.ascontiguousarray(wq2).astype(bf),
            "wo2": np.ascontiguousarray(wo2.reshape(128, 4 * HC * 128)).astype(bf),
            "permT": permT,
            "cos2": np.ascontiguousarray(
                np.tile(cos_t[pos[b]].T, (2, 1))).astype(bf),
            "sin2": np.ascontiguousarray(
                np.tile(sin_t[pos[b]].T, (2, 1))).astype(bf),
        }
        if use_mask:
            mt = np.maximum(mask[b, 0], NEG_BIG).T    # [S(k), S(q)]
            m["maskT"] = np.ascontiguousarray(mt).astype(bf)
        in_maps.append(m)

    res = run_bass_kernel_spmd(nc, in_maps, core_ids=list(range(8)),
                               trace=_trace)
    LAST_RESULT = res
    out = np.zeros((B, S, HID), dtype=np.float32)
    for c in range(8):
        out[c // G] += res.results[c]["out"].astype(np.float32).T
    return out


# revision 6
# speedup vs baseline: 1.0346x; 1.0346x over previous
"""GQA attention (B=2,S=1024,HID=2048,NH=32,NKV=8,HD=64) on 8 TRN2 cores — v4.

Sharding: core c -> batch b=c//4, head-group g=c%4 (8 q heads / 2 kv heads).
Per core: partial out[b] = attn(heads of g) @ Wo[rows of g]; host sums the 4
row-parallel bf16 partials per batch.

The exp stream on ACT (64 x [128,1024] ~= 66us) and total PE work (~125us)
are the two walls; the schedule keeps both streams dense:

  phase A: ONE k-chunk-outer chase loop computes kv-proj, natural-layout V,
           AND q-chunk mc0 per hsT chunk (12 matmuls ~= 1.3us PE per 1.06us
           of DMA), fed by a host-packed per-chunk weight block
           [wk_k|wv_k|wq0_k] interleaved with the hsT chunks in DMA order.
           Then mc1 + the kv/mc0 rope rotations interleaved.
  phase B: attention slot stream (h,kc): sc(i) -> exp(i) -> fill -> pv(i-1).
           scores psum bufs=2 keeps the exp cadence gapless; PV lags one
           slot so it never head-of-line blocks the PE behind an exp wait;
           rope(mc1) + mc2/mc3 projections + their ropes are drip-fed as
           filler (~0.9 steps/slot). ps_at (PV accum, bufs=1) is drained to
           SBUF by a Pool-engine copy (DVE is busy; Pool is the fast lane)
           right after pv(h,7); softmax normalize (recip -> row-0 hop DMA ->
           gpsimd broadcast -> mult) runs off-psum. All heads use augmented
           [V|1] PV weights: denominators land on psum row 64 (no
           ones-matmuls). Odd heads reach attnT rows 64..127 via SBUF DMA.
  phase C: Wo with kc2=3 (head 6/7 rows) emitted two mc2-passes late so the
           head-7 normalize hides under the first Wo passes; psum->SBUF
           output copies alternate ACT/DVE; bf16 output halves the out-DMA.

PSUM banks: A: pA 3x[128,1024] + rot 1x = 8; B: stp 2x + avp 1x + pB 1x = 8;
C: wop 3x = 6.
"""

import numpy as np
import ml_dtypes

import concourse.bass as bass
import concourse.bacc as bacc
import concourse.mybir as mybir
from concourse.tile import TileContext
from concourse.bass_utils import run_bass_kernel_spmd
from concourse.masks import make_identity

B, S, HID = 2, 1024, 2048
NH, NKV, HD = 32, 8, 64
G = 4                      # head groups (tensor-parallel degree per batch)
QH = NH // G               # 8 q heads per core
KVH = NKV // G             # 2 kv heads per core
QD = QH * HD               # 512
HC = HID // 128            # 16 hidden chunks
TC = S // 128              # 8 token chunks
KC = S // 128              # 8 k chunks
ROPE_BASE = 10000.0
BF16 = mybir.dt.bfloat16
F32 = mybir.dt.float32
NEG_BIG = float(np.finfo(np.float32).min)

LAST_RESULT = None
_CACHE = {}


def _build(use_mask: bool) -> bass.Bass:
    nc = bacc.Bacc(None, target_bir_lowering=False)
    hsT_d = nc.dram_tensor("hsT", [HID, S], BF16, kind="ExternalInput")
    # per-chunk weight block: [wk_k (128) | wv_k (128) | wq_mc0_k (128)]
    wch_d = nc.dram_tensor("wch", [128, HC * 384], BF16, kind="ExternalInput")
    wq_d = nc.dram_tensor("wq2", [128, 3 * HC * 128], BF16, kind="ExternalInput")
    wo_d = nc.dram_tensor("wo2", [128, 4 * HC * 128], BF16, kind="ExternalInput")
    cos_d = nc.dram_tensor("cos2", [128, S], BF16, kind="ExternalInput")
    sin_d = nc.dram_tensor("sin2", [128, S], BF16, kind="ExternalInput")
    perm_d = nc.dram_tensor("permT", [128, 128], BF16, kind="ExternalInput")
    if use_mask:
        mask_d = nc.dram_tensor("maskT", [S, S], BF16, kind="ExternalInput")
    out_d = nc.dram_tensor("out", [HID, S], BF16, kind="ExternalOutput")

    with TileContext(nc) as tc:
        with (
            tc.tile_pool(name="resid", bufs=1) as rp,
            tc.tile_pool(name="ex", bufs=3) as ep,
            tc.tile_pool(name="wk_", bufs=2) as wp,
            tc.tile_pool(name="outs", bufs=4) as op_,
        ):
            # ---- resident SBUF tiles + input DMA (SP order == emission) ----
            wch = rp.tile([128, HC * 384], BF16, tag="wch")
            hsT = []
            for k in range(HC):
                nc.sync.dma_start(out=wch[:, k * 384:(k + 1) * 384],
                                  in_=wch_d[:, k * 384:(k + 1) * 384])
                t = rp.tile([128, S], BF16, tag=f"hsT{k}")
                if k == 0:
                    nc.sync.dma_start(out=t[:, 0:512],
                                      in_=hsT_d[0:128, 0:512])
                    nc.sync.dma_start(out=t[:, 512:1024],
                                      in_=hsT_d[0:128, 512:1024])
                else:
                    nc.sync.dma_start(out=t[:],
                                      in_=hsT_d[k * 128:(k + 1) * 128, :])
                hsT.append(t)
            wq2 = rp.tile([128, 3 * HC * 128], BF16, tag="wq2")
            nc.sync.dma_start(out=wq2[:, 0:2048], in_=wq_d[:, 0:2048])
            permT = rp.tile([128, 128], BF16, tag="permT")
            nc.sync.dma_start(out=permT[:], in_=perm_d[:, :])
            cos2 = rp.tile([128, S], BF16, tag="cos2")
            nc.sync.dma_start(out=cos2[:], in_=cos_d[:, :])
            sin2 = rp.tile([128, S], BF16, tag="sin2")
            nc.sync.dma_start(out=sin2[:], in_=sin_d[:, :])
            nc.sync.dma_start(out=wq2[:, 2048:4096], in_=wq_d[:, 2048:4096])
            nc.sync.dma_start(out=wq2[:, 4096:6144], in_=wq_d[:, 4096:6144])
            if use_mask:
                maskT = rp.tile([128, KC * S], BF16, tag="maskT")
                nc.sync.dma_start(
                    out=maskT[:].rearrange("p (k q) -> p k q", k=KC),
                    in_=mask_d[:, :].rearrange("(k p) q -> p k q", p=128),
                )
            wo2 = rp.tile([128, 4 * HC * 128], BF16, tag="wo2")

            # ---- persistent intermediates ----
            qrot = rp.tile([128, 4 * S], BF16, tag="qrot")
            krot = rp.tile([128, S], BF16, tag="krot")
            krep = rp.tile([128, KVH * S], BF16, tag="krep")
            # vaug per (kc, kv): 128 cols [1 | 0*63 | V(64)] — PV lhsT puts
            # the softmax denominator on psum row 0 (so partition_broadcast
            # reads partition 0, the HW-supported source) and attn on rows
            # 64..127 (a DVE-legal 64-partition window).
            vaug = rp.tile([128, KC * KVH * 128], BF16, tag="vaug")
            nc.any.memset(vaug[:], 0.0)
            ones_cols = vaug[:].rearrange("p (b c) -> p b c", c=128)[:, :, 0:1]
            nc.vector.memset(ones_cols, 1.0)
            attnT = [rp.tile([128, S], BF16, tag=f"attnT{i}", name=f"attnT{i}")
                     for i in range(4)]

            # preload the exp activation table while ACT is idle
            junk = rp.tile([1, 4], F32, tag="junk")
            nc.vector.memset(junk[:], 0.0)
            nc.scalar.activation(
                junk[0:1, 2:4], junk[0:1, 0:2],
                mybir.ActivationFunctionType.Exp
            )

            t1 = rp.tile([128, S], BF16, tag="t1")
            t2 = rp.tile([128, S], BF16, tag="t2")
            ident = rp.tile([128, 128], BF16, tag="ident")
            make_identity(nc, ident[:])

            def rope_combine(raw, ps_rot, dst):
                """dst = raw*cos + rot(raw)*sin, all [128, S] on DVE."""
                nc.vector.tensor_tensor(t1[:], raw[:], cos2[:],
                                        mybir.AluOpType.mult)
                nc.vector.tensor_tensor(t2[:], ps_rot[:], sin2[:],
                                        mybir.AluOpType.mult)
                nc.vector.tensor_tensor(dst, t1[:], t2[:], mybir.AluOpType.add)

            # ===== phase A: kv+v+mc0 in one DMA-chased loop, then mc1 =====
            with (
                tc.tile_pool(name="pA", bufs=3, space="PSUM") as pA,
                tc.tile_pool(name="rot", bufs=1, space="PSUM") as rot_p,
            ):
                ps_kv = pA.tile([128, S], F32, tag="pA")
                ps_vt = pA.tile([128, S], F32, tag="pA")
                ps_q0 = pA.tile([128, S], F32, tag="pA")
                for k in range(HC):
                    wb = k * 384
                    for ns in range(2):
                        nc.tensor.matmul(
                            ps_kv[:, ns * 512:(ns + 1) * 512],
                            wch[:, wb:wb + 128],
                            hsT[k][:, ns * 512:(ns + 1) * 512],
                            start=(k == 0), stop=(k == HC - 1),
                        )
                    for ns in range(2):
                        nc.tensor.matmul(
                            ps_vt[:, ns * 512:(ns + 1) * 512],
                            wch[:, wb + 128:wb + 256],
                            hsT[k][:, ns * 512:(ns + 1) * 512],
                            start=(k == 0), stop=(k == HC - 1),
                        )
                    for ns in range(2):
                        nc.tensor.matmul(
                            ps_q0[:, ns * 512:(ns + 1) * 512],
                            wch[:, wb + 256:wb + 384],
                            hsT[k][:, ns * 512:(ns + 1) * 512],
                            start=(k == 0), stop=(k == HC - 1),
                        )

                # psum -> sbuf raws on ACT (run during mc1 below)
                kraw = rp.tile([128, S], BF16, tag="kraw")
                nc.scalar.activation(
                    kraw[:], ps_kv[:], mybir.ActivationFunctionType.Copy
                )
                q0raw = rp.tile([128, S], BF16, tag="q0raw")
                nc.scalar.activation(
                    q0raw[:], ps_q0[:], mybir.ActivationFunctionType.Copy
                )
                vt_sb = rp.tile([128, S], BF16, tag="vt_sb")
                nc.scalar.activation(
                    vt_sb[:], ps_vt[:], mybir.ActivationFunctionType.Copy
                )

                # mc1 projection, with the kv/mc0 rope rotations interleaved
                ps_q1 = pA.tile([128, S], F32, tag="pA")

                def mc1_half(lo, hi):
                    for k in range(lo, hi):
                        for ns in range(2):
                            nc.tensor.matmul(
                                ps_q1[:, ns * 512:(ns + 1) * 512],
                                wq2[:, k * 128:(k + 1) * 128],
                                hsT[k][:, ns * 512:(ns + 1) * 512],
                                start=(k == 0), stop=(k == HC - 1),
                            )

                mc1_half(0, 4)
                ps_r0 = rot_p.tile([128, S], F32, tag="rot")
                for ns in range(2):
                    nc.tensor.matmul(
                        ps_r0[:, ns * 512:(ns + 1) * 512],
                        permT[:], q0raw[:, ns * 512:(ns + 1) * 512],
                        start=True, stop=True,
                    )
                # PE-transpose V^T chunks into natural [token, dim] layout;
                # each transpose is its own start+stop group (bank-legal)
                trt = pA.tile([128, 2 * S], BF16, tag="pA", name="trt")
                for t in range(TC):
                    nc.tensor.transpose(
                        trt[:, t * 128:(t + 1) * 128],
                        vt_sb[:, t * 128:(t + 1) * 128], ident[:]
                    )
                # t1 products need only the raws: run them on DVE while the
                # rot matmuls are still in flight
                t1k = rp.tile([128, S], BF16, tag="t1k")
                nc.vector.tensor_tensor(t1k[:], kraw[:], cos2[:],
                                        mybir.AluOpType.mult)
                nc.vector.tensor_tensor(t1[:], q0raw[:], cos2[:],
                                        mybir.AluOpType.mult)
                ps_rk = pA.tile([128, S], F32, tag="pA", name="ps_rk")
                for ns in range(2):
                    nc.tensor.matmul(
                        ps_rk[:, ns * 512:(ns + 1) * 512],
                        permT[:], kraw[:, ns * 512:(ns + 1) * 512],
                        start=True, stop=True,
                    )
                mc1_half(4, 6)
                t2k = rp.tile([128, S], BF16, tag="t2k")
                nc.vector.tensor_tensor(t2k[:], ps_rk[:], sin2[:],
                                        mybir.AluOpType.mult)
                nc.vector.tensor_tensor(krot[:], t1k[:], t2k[:],
                                        mybir.AluOpType.add)
                # krep: kv head i duplicated into both 64-row halves
                nc.vector.tensor_copy(krep[0:64, 0:S], krot[0:64, :])
                nc.vector.tensor_copy(krep[64:128, S:2 * S], krot[64:128, :])
                nc.sync.dma_start(out=krep[64:128, 0:S], in_=krot[0:64, :])
                nc.sync.dma_start(out=krep[0:64, S:2 * S], in_=krot[64:128, :])
                nc.vector.tensor_tensor(t2[:], ps_r0[:], sin2[:],
                                        mybir.AluOpType.mult)
                nc.vector.tensor_tensor(qrot[:, 0:S], t1[:], t2[:],
                                        mybir.AluOpType.add)
                mc1_half(6, 11)
                mc1_half(11, HC)

                # vaug: V cols at base+64..base+127 per (t, kv) — on ACT
                # (idle in phase A; keeps the DVE sem counts that the first
                # scores' merged waits reference free of vaug traffic)
                for t in range(TC):
                    for kv in range(KVH):
                        base = (t * KVH + kv) * 128
                        nc.scalar.activation(
                            vaug[:, base + 64:base + 128],
                            trt[:, t * 128 + kv * 64:t * 128 + kv * 64 + 64],
                            mybir.ActivationFunctionType.Copy,
                        )

                q1raw = rp.tile([128, S], BF16, tag="q1raw")
                nc.scalar.activation(
                    q1raw[:], ps_q1[:], mybir.ActivationFunctionType.Copy
                )

            # ===== phase B: attention slot stream with rope(mc1)+mc2/mc3 fill
            with (
                tc.tile_pool(name="st", bufs=2, space="PSUM") as stp,
                tc.tile_pool(name="av", bufs=1, space="PSUM") as avp,
                tc.tile_pool(name="pB", bufs=1, space="PSUM") as pBp,
            ):
                def fill_steps():
                    # rope(mc1) first: needed by h2 (slot 16)
                    ps_r1 = pBp.tile([128, S], F32, tag="pB")
                    for ns in range(2):
                        nc.tensor.matmul(
                            ps_r1[:, ns * 512:(ns + 1) * 512],
                            permT[:], q1raw[:, ns * 512:(ns + 1) * 512],
                            start=True, stop=True,
                        )
                    yield
                    rope_combine(q1raw, ps_r1, qrot[:, S:2 * S])
                    yield
                    # deferred mc2/mc3 projection + rope, ~2 matmuls/step
                    for mc in (2, 3):
                        ps_q = pBp.tile([128, S], F32, tag="pB")
                        for k in range(HC):
                            for ns in range(2):
                                nc.tensor.matmul(
                                    ps_q[:, ns * 512:(ns + 1) * 512],
                                    wq2[:, ((mc - 1) * HC + k) * 128:
                                        ((mc - 1) * HC + k + 1) * 128],
                                    hsT[k][:, ns * 512:(ns + 1) * 512],
                                    start=(k == 0), stop=(k == HC - 1),
                                )
                            yield
                        qraw = rp.tile([128, S], BF16, tag=f"qraw{mc}")
                        nc.vector.tensor_copy(qraw[:], ps_q[:])  # ACT is busy
                        yield
                        ps_rq = pBp.tile([128, S], F32, tag="pB")
                        for ns in range(2):
                            nc.tensor.matmul(
                                ps_rq[:, ns * 512:(ns + 1) * 512],
                                permT[:], qraw[:, ns * 512:(ns + 1) * 512],
                                start=True, stop=True,
                            )
                        yield
                        rope_combine(qraw, ps_rq, qrot[:, mc * S:(mc + 1) * S])
                        yield

                fill_gen = fill_steps()

                def fill(n=1):
                    nonlocal fill_gen
                    for _ in range(n):
                        if fill_gen is None:
                            return
                        try:
                            next(fill_gen)
                        except StopIteration:
                            fill_gen = None
                            return

                def emit_sc(h, kc):
                    kv = h // 4
                    mc = h // 2
                    r = (h % 2) * 64
                    ps_st = stp.tile([128, S], F32, tag="st")
                    for ns in range(2):
                        nc.tensor.matmul(
                            ps_st[:, ns * 512:(ns + 1) * 512],
                            krep[r:r + 64,
                                 kv * S + kc * 128:kv * S + (kc + 1) * 128],
                            qrot[r:r + 64,
                                 mc * S + ns * 512:mc * S + ns * 512 + 512],
                            start=True, stop=True,
                        )
                    if use_mask:
                        nc.vector.tensor_tensor(
                            ps_st[:], ps_st[:], maskT[:, kc * S:(kc + 1) * S],
                            mybir.AluOpType.add,
                        )
                    ex = ep.tile([128, S], BF16, tag="ex")
                    nc.scalar.activation(
                        ex[:], ps_st[:], mybir.ActivationFunctionType.Exp
                    )
                    return ex

                at_tiles = {}

                def emit_pv(h, kc, ex):
                    kv = h // 4
                    if kc == 0:
                        at_tiles[h] = avp.tile([128, S], F32, tag="av",
                                               name=f"av{h}")
                    ps_at = at_tiles[h]
                    base = (kc * KVH + kv) * 65
                    for ns in range(2):
                        nc.tensor.matmul(
                            ps_at[0:65, ns * 512:(ns + 1) * 512],
                            vaug[:, base:base + 65],
                            ex[:, ns * 512:(ns + 1) * 512],
                            start=(kc == 0), stop=(kc == KC - 1),
                        )

                def emit_norm(h):
                    """Drain ps_at to SBUF on Pool (fast lane), normalize
                    off-psum: recip -> row-0 hop -> broadcast -> mult."""
                    mc = h // 2
                    ps_at = at_tiles.pop(h)
                    atc = wp.tile([128, S], F32, tag="atc")
                    nc.gpsimd.tensor_copy(atc[0:65, :], ps_at[0:65, :])
                    nc.vector.reciprocal(atc[64:65, :], atc[64:65, :])
                    bcast = wp.tile([128, S], F32, tag="bcast")
                    nc.gpsimd.partition_broadcast(bcast[0:64, :], atc[64:65, :])
                    if h % 2 == 0:
                        nc.vector.tensor_tensor(
                            attnT[mc][0:64, :],
                            atc[0:64, :], bcast[0:64, :],
                            mybir.AluOpType.mult,
                        )
                    else:
                        tmp = wp.tile([128, S], BF16, tag="otmp")
                        nc.vector.tensor_tensor(
                            tmp[0:64, :], atc[0:64, :], bcast[0:64, :],
                            mybir.AluOpType.mult,
                        )
                        nc.sync.dma_start(
                            out=attnT[mc][64:128, :],
                            in_=tmp[0:64, :],
                        )

                slots = [(h, kc) for h in range(QH) for kc in range(KC)]
                prev = None
                for i, (h, kc) in enumerate(slots):
                    ex = emit_sc(h, kc)
                    if i % 8 != 7:
                        fill(1)
                    if prev is not None:
                        emit_pv(*prev)
                        if prev[1] == KC - 1:
                            emit_norm(prev[0])
                    prev = (h, kc, ex)
                emit_pv(*prev)
                emit_norm(prev[0])
                fill(100)

            # ===== phase C: output projection (transposed out, bf16) =====
            # kc2=3 (heads 6/7) is emitted two mc2-passes late so the first Wo
            # passes overlap the head-7 normalize chain.
            with tc.tile_pool(name="wop", bufs=3, space="PSUM") as wop:
                pso_t = {}

                def wo_part(mc2, kc2s, start, stop):
                    if mc2 not in pso_t:
                        pool = wop2 if mc2 % 3 == 0 else wop
                        pso_t[mc2] = pool.tile([128, S], F32, tag="wop",
                                               name=f"wop{mc2}")
                    pso = pso_t[mc2]
                    for ns in range(2):
                        for kc2 in kc2s:
                            nc.tensor.matmul(
                                pso[:, ns * 512:(ns + 1) * 512],
                                wo2[:, (kc2 * HC + mc2) * 128:
                                    (kc2 * HC + mc2 + 1) * 128],
                                attnT[kc2][:, ns * 512:ns * 512 + 512],
                                start=(start and kc2 == kc2s[0]),
                                stop=(stop and kc2 == kc2s[-1]),
                            )

                def wo_finish(mc2, split=False):
                    wo_part(mc2, [3], start=False, stop=True)
                    pso = pso_t.pop(mc2)
                    osb = op_.tile([128, S], BF16, tag="osb")
                    if split:
                        # drain halves on both engines in parallel
                        nc.scalar.activation(
                            osb[:, 0:512], pso[:, 0:512],
                            mybir.ActivationFunctionType.Copy
                        )
                        nc.vector.tensor_copy(osb[:, 512:1024],
                                              pso[:, 512:1024])
                        for j in range(2):
                            sl = slice(j * 512, (j + 1) * 512)
                            nc.sync.dma_start(
                                out=out_d[mc2 * 128:(mc2 + 1) * 128, sl],
                                in_=osb[:, sl],
                            )
                    elif mc2 % 2 == 0:
                        nc.scalar.activation(
                            osb[:], pso[:], mybir.ActivationFunctionType.Copy
                        )
                        nc.sync.dma_start(
                            out=out_d[mc2 * 128:(mc2 + 1) * 128, :], in_=osb[:]
                        )
                    else:
                        nc.vector.tensor_copy(osb[:], pso[:])
                        nc.sync.dma_start(
                            out=out_d[mc2 * 128:(mc2 + 1) * 128, :], in_=osb[:]
                        )

                NW = HID // 128
                for mc2 in (1, 2, 3):
                    wo_part(mc2, [0, 1, 2], start=True, stop=False)
                wo_finish(0)
                wo_finish(1)
                for mc2 in range(4, NW):
                    wo_part(mc2, [0, 1, 2], start=True, stop=False)
                    wo_finish(mc2 - 2)
                wo_finish(NW - 2, split=True)
                wo_finish(NW - 1, split=True)
    nc.finalize()
    return nc


def _rope_tables():
    inv = 1.0 / (ROPE_BASE ** (np.arange(0, HD, 2, dtype=np.float32) / HD))
    t = np.arange(S, dtype=np.float32)
    freqs = np.outer(t, inv)
    emb = np.concatenate([freqs, freqs], axis=-1)  # [S, HD]
    return np.cos(emb).astype(np.float32), np.sin(emb).astype(np.float32)


def _perm_T():
    P = np.zeros((128, 128), dtype=np.float32)
    for blk in range(2):
        o = blk * 64
        for i in range(32):
            P[o + i, o + i + 32] = -1.0
            P[o + i + 32, o + i] = 1.0
    return P.T.astype(ml_dtypes.bfloat16)


def kernel(hidden_states, position_ids, attention_mask, Wq, Wk, Wv, Wo,
           _trace=False):
    global LAST_RESULT
    bf = ml_dtypes.bfloat16
    hidden_states = np.asarray(hidden_states, dtype=np.float32)
    Wq = np.asarray(Wq, dtype=np.float32)
    Wk = np.asarray(Wk, dtype=np.float32)
    Wv = np.asarray(Wv, dtype=np.float32)
    Wo = np.asarray(Wo, dtype=np.float32)
    mask = np.asarray(attention_mask, dtype=np.float32)
    pos = np.asarray(position_ids).astype(np.int64)

    use_mask = bool(np.any(mask))
    if use_mask not in _CACHE:
        _CACHE[use_mask] = _build(use_mask)
    nc = _CACHE[use_mask]

    cos_t, sin_t = _rope_tables()
    permT = _perm_T()
    scale = 1.0 / np.sqrt(HD)

    in_maps = []
    for c in range(8):
        b, g = c // G, c % G
        wq_g = (Wq[:, g * QD:(g + 1) * QD] * scale).astype(np.float32)
        # [k, p, mc, c] -> per-chunk [p, k, (wk|wv|wq0)] and mc-major rest
        wq4 = wq_g.reshape(HC, 128, 4, 128)
        wk3 = Wk[:, g * 128:(g + 1) * 128].reshape(HC, 128, 128)
        wv3 = Wv[:, g * 128:(g + 1) * 128].reshape(HC, 128, 128)
        wch = np.concatenate([wk3, wv3, wq4[:, :, 0]], axis=2)  # [k, p, 384]
        wch = wch.transpose(1, 0, 2).reshape(128, HC * 384)
        wq2 = wq4[:, :, 1:].transpose(1, 2, 0, 3).reshape(128, 3 * HC * 128)
        wo2 = Wo[g * QD:(g + 1) * QD, :].reshape(4, 128, HC, 128).transpose(1, 0, 2, 3)
        m = {
            "hsT": np.ascontiguousarray(hidden_states[b].T).astype(bf),
            "wch": np.ascontiguousarray(wch).astype(bf),
            "wq2": np.ascontiguousarray(wq2).astype(bf),
            "wo2": np.ascontiguousarray(wo2.reshape(128, 4 * HC * 128)).astype(bf),
            "permT": permT,
            "cos2": np.ascontiguousarray(
                np.tile(cos_t[pos[b]].T, (2, 1))).astype(bf),
            "sin2": np.ascontiguousarray(
                np.tile(sin_t[pos[b]].T, (2, 1))).astype(bf),
        }
        if use_mask:
            mt = np.maximum(mask[b, 0], NEG_BIG).T    # [S(k), S(q)]
            m["maskT"] = np.ascontiguousarray(mt).astype(bf)
        in_maps.append(m)

    res = run_bass_kernel_spmd(nc, in_maps, core_ids=list(range(8)),
                               trace=_trace)
    LAST_RESULT = res
    out = np.zeros((B, S, HID), dtype=np.float32)
    for c in range(8):
        out[c // G] += res.results[c]["out"].astype(np.float32).T
    return out
